# revision 20
# baseline (speedup 1.0000x reference)
"""Trainium2 Bass kernel: fused bmm+decay+reduce attention scorer (bf16 v5).

Computes, for full inputs
    self_attn  [N=16, M=100, EMB=128] f32
    self_delta [N=16, M=100, L=10000, D=4] f32
    emb_table  [L+1=10001, EMB=128] f32
    value_w    [M=100] f32
the output
    out[n, l] = sum_m value_w[m] * (sum_d self_delta[n,m,l,d]) * (emb_table[1+l] . self_attn[n,m])
of shape [16, 10000] f32 (matches the reference jnp einsum chain).

Sharding: the candidate/location axis L is split 8 ways (1250 locations per
core); every core handles all 16 batch rows for its location range, so the
dominant self_delta stream is never replicated.

Structure (130 us f32 baseline -> 75 us v4 -> this):
  *  The delta stream is staged host-side as bf16 (graded check is
     rel-err < 2e-2; measured impact ~4.6e-3), halving HBM traffic to
     16.7 MB/core, d-plane-major + l-padded to 1254 = 3*418 so every DVE
     operand is a packed 2-byte stream (DVE 2x perf mode).
  *  S = emb.attn is evacuated PSUM->SBUF with bf16 downcast on the ACT
     engine, so the decay multiply is an all-bf16 2x-mode op too.
  *  The raw stream lands in a fully resident 13-buffer pool; all DMAs
     are issued up-front on ONE HWDGE ring (SP) and sustain ~390 GB/s.
     A single in-order ring matters: v5 tried alternating two rings and
     every early tile arrived ~2x later (each ring runs at half rate),
     delaying the whole in-order compute pipeline.
  *  Consts are issued at the head of the same ring (in-order completion
     guarantees they land before the raw flood; v2 showed they starve
     on a parallel ring and stall PE until 17 us).
  *  The FIRST and LAST tiles are processed in three 418-wide l-chunks
     with per-chunk DMA, adds and multiply (plus per-chunk acc-matmul,
     PSUM-evacuation and output DMA for the last), pulling the pipeline
     start ~1 us earlier and shrinking the post-stream drain tail from
     ~8 us to ~4 us of pipelined chunk work.
Roofline: 16.9 MB at ~390 GB/s ~= 43 us of stream + ~7.9 us fixed
preamble + ~4 us tail + ~8.6 us fixed NEFF semaphore-reset postamble.
"""

import numpy as np
import ml_dtypes

import concourse.mybir as mybir
import concourse.tile as tile
from concourse import bacc
from concourse.bass_utils import run_bass_kernel_spmd

BF16NP = ml_dtypes.bfloat16

N, M, L, EMB, D = 16, 100, 10000, 128, 4
NCORES = 8
LSH = L // NCORES  # 1250 locations per core
LP = 1254          # padded to 3 * 418 (uniform PSUM-bank chunks, even widths)
CW = 418           # chunk width (<=512 f32 per PSUM bank)
NCHUNK = 3
R = N * M          # 1600 flattened (n, m) rows
P = 128
NTILE = (R + P - 1) // P  # 13 tiles; the last holds 64 rows
RP = NTILE * P            # 1664 padded rows
TILE_ORDER = [NTILE - 1] + list(range(NTILE - 1))  # half tile first (short ramp)
FP32 = mybir.dt.float32
BF16 = mybir.dt.bfloat16

_NC_CACHE = {}


def _build_nc():
    nc = bacc.Bacc(
        "TRN2", target_bir_lowering=False, debug=False, num_devices=NCORES
    )
    raw_d = nc.dram_tensor("raw", [RP, D, LP], BF16, kind="ExternalInput").ap()
    embT_d = nc.dram_tensor("embT", [EMB, LP], BF16, kind="ExternalInput").ap()
    attnT_d = nc.dram_tensor("attnT", [EMB, RP], BF16, kind="ExternalInput").ap()
    vwoh_d = nc.dram_tensor("vwoh", [P, NTILE * N], BF16, kind="ExternalInput").ap()
    out_d = nc.dram_tensor("out", [N, LSH], FP32, kind="ExternalOutput").ap()

    with tile.TileContext(nc) as tc:
        with (
            tc.tile_pool(name="const", bufs=1) as cpool,
            tc.tile_pool(name="raws", bufs=NTILE) as rpool,
            tc.tile_pool(name="a1p", bufs=2) as a1pool,
            tc.tile_pool(name="a2p", bufs=4) as a2pool,
            tc.tile_pool(name="ssb", bufs=4) as sbpool,
            tc.tile_pool(name="ptp", bufs=3) as ppool,
            tc.tile_pool(name="spsum", bufs=1, space="PSUM") as spool,
            tc.tile_pool(name="apsum", bufs=1, space="PSUM") as apool,
        ):
            # out accumulator rows n=0..15, one PSUM bank per l-chunk
            acc = apool.tile([N, NCHUNK, 512], FP32, tag="acc")

            # all raw-tile DMAs are independent of compute: issue the whole
            # stream up-front on one in-order ring so it never stalls and
            # tiles arrive in consumption order.  The first and last tiles
            # are transferred as three l-chunks to chunk-pipeline the
            # pipeline ramp and the drain tail.  The first two tiles go
            # BEFORE the consts (DVE needs raw ~3 us before PE needs consts).
            # chunked tiles: the first two (DVE otherwise stalls ~2 us
            # waiting whole-tile arrivals behind the consts) and the last
            # (chunk-pipelined drain tail)
            chunked = {TILE_ORDER[0], TILE_ORDER[1], TILE_ORDER[-1]}
            raws = {}

            def emit_raw_dma(t):
                rows = min(P, R - t * P)
                raw_t = rpool.tile([P, D, LP], BF16, tag="raw")
                if t in chunked:
                    for j in range(NCHUNK):
                        nc.sync.dma_start(
                            out=raw_t[:rows, :, j * CW : (j + 1) * CW],
                            in_=raw_d[
                                t * P : t * P + rows, :, j * CW : (j + 1) * CW
                            ],
                        )
                else:
                    nc.sync.dma_start(
                        out=raw_t[:rows], in_=raw_d[t * P : t * P + rows]
                    )
                raws[t] = raw_t

            emit_raw_dma(TILE_ORDER[0])
            embT = cpool.tile([EMB, LP], BF16, tag="embT")
            nc.sync.dma_start(out=embT, in_=embT_d)
            attnT = cpool.tile([EMB, RP], BF16, tag="attnT")
            nc.sync.dma_start(out=attnT, in_=attnT_d)
            vwoh = cpool.tile([P, NTILE * N], BF16, tag="vwoh")
            nc.sync.dma_start(out=vwoh, in_=vwoh_d)
            for t in TILE_ORDER[1:]:
                emit_raw_dma(t)

            out_sb = cpool.tile([N, NCHUNK, CW], FP32, tag="out_sb")
            tiles = {}

            def emit_feed(t, adds_chunked):
                """S matmuls + ACT evacuation + the two D-sum adds."""
                rows = min(P, R - t * P)
                rv = raws[t]
                s_ps = spool.tile([P, NCHUNK, 512], FP32, tag="s")
                for j in range(NCHUNK):
                    nc.tensor.matmul(
                        s_ps[:rows, j, :CW],
                        attnT[:, t * P : t * P + rows],
                        embT[:, j * CW : (j + 1) * CW],
                        start=True,
                        stop=True,
                    )
                # evacuate + downcast on the ACT engine (keeps DVE free)
                s_sb = sbpool.tile([P, NCHUNK, CW], BF16, tag="ssb")
                nc.scalar.copy(out=s_sb[:rows], in_=s_ps[:rows, :, :CW])
                a1 = a1pool.tile([P, 2, LP], BF16, tag="a1")
                a2 = a2pool.tile([P, LP], BF16, tag="a2")
                p_t = ppool.tile([P, NCHUNK, CW], BF16, tag="p")
                tiles[t] = (rows, a2, s_sb, p_t)
                for j in range(NCHUNK) if adds_chunked else [None]:
                    emit_adds(t, rows, rv, a1, a2, j)

            def emit_adds(t, rows, rv, a1, a2, j):
                lsl = slice(None) if j is None else slice(j * CW, (j + 1) * CW)
                nc.vector.tensor_add(
                    out=a1[:rows, :, lsl],
                    in0=rv[:rows, 0:2, lsl],
                    in1=rv[:rows, 2:4, lsl],
                )
                nc.vector.tensor_add(
                    out=a2[:rows, lsl],
                    in0=a1[:rows, 0, lsl],
                    in1=a1[:rows, 1, lsl],
                )

            def emit_drain(t, ti, j):
                """Decay multiply + acc matmul(s) for tile t (chunk j or all)."""
                rows, a2, s_sb, p_t = tiles[t]
                a2v = a2.rearrange("p (c w) -> p c w", w=CW)
                csl = slice(None) if j is None else slice(j, j + 1)
                nc.vector.tensor_mul(
                    out=p_t[:rows, csl], in0=a2v[:rows, csl], in1=s_sb[:rows, csl]
                )
                last = ti == NTILE - 1
                for jj in range(NCHUNK) if j is None else [j]:
                    nc.tensor.matmul(
                        acc[:, jj, :CW],
                        vwoh[:rows, t * N : (t + 1) * N],
                        p_t[:rows, jj],
                        start=(ti == 0),
                        stop=last,
                    )
                    if last:
                        # chunk jj of acc is final: evacuate + store now,
                        # overlapped with the next chunk's DVE/PE work
                        w = min(LSH, (jj + 1) * CW) - jj * CW
                        nc.scalar.copy(out=out_sb[:, jj], in_=acc[:, jj, :CW])
                        nc.sync.dma_start(
                            out=out_d[:, jj * CW : jj * CW + w],
                            in_=out_sb[:, jj, :w],
                        )

            # two-tile software pipeline: tile t's multiply is emitted after
            # tile t+2's adds, so the in-order DVE queue never blocks on the
            # PE->ACT S path (measured 2.1 us stall at depth 1); the last
            # tile interleaves per chunk to keep the drain tail short
            KDEPTH = 2
            for ti, t in enumerate(TILE_ORDER):
                last = ti == NTILE - 1
                if not last:
                    emit_feed(t, adds_chunked=(t in chunked))
                    if ti >= KDEPTH:
                        emit_drain(TILE_ORDER[ti - KDEPTH], ti - KDEPTH, None)
                else:
                    for k in range(KDEPTH, 0, -1):
                        emit_drain(TILE_ORDER[ti - k], ti - k, None)
                    rows = min(P, R - t * P)
                    rv = raws[t]
                    s_ps = spool.tile([P, NCHUNK, 512], FP32, tag="s")
                    for j in range(NCHUNK):
                        nc.tensor.matmul(
                            s_ps[:rows, j, :CW],
                            attnT[:, t * P : t * P + rows],
                            embT[:, j * CW : (j + 1) * CW],
                            start=True,
                            stop=True,
                        )
                    s_sb = sbpool.tile([P, NCHUNK, CW], BF16, tag="ssb")
                    nc.scalar.copy(out=s_sb[:rows], in_=s_ps[:rows, :, :CW])
                    a1 = a1pool.tile([P, 2, LP], BF16, tag="a1")
                    a2 = a2pool.tile([P, LP], BF16, tag="a2")
                    p_t = ppool.tile([P, NCHUNK, CW], BF16, tag="p")
                    tiles[t] = (rows, a2, s_sb, p_t)
                    for j in range(NCHUNK):
                        emit_adds(t, rows, rv, a1, a2, j)
                        emit_drain(t, ti, j)

    nc.compile()
    return nc


def _get_nc():
    if "nc" not in _NC_CACHE:
        _NC_CACHE["nc"] = _build_nc()
    return _NC_CACHE["nc"]


def _prep_in_maps(self_attn, self_delta, emb_table, value_w):
    self_attn = np.asarray(self_attn, dtype=np.float32)
    self_delta = np.asarray(self_delta, dtype=np.float32)
    emb_table = np.asarray(emb_table, dtype=np.float32)
    value_w = np.asarray(value_w, dtype=np.float32)

    # [R, D, L] bf16, d-plane-major: one global transpose+cast, sliced per core
    raw_all = np.ascontiguousarray(
        self_delta.reshape(R, L, D).transpose(0, 2, 1)
    ).astype(BF16NP)

    embT_full = emb_table[1 : L + 1].T.astype(BF16NP)  # [EMB, L]

    # column r = n*M + m of attnT holds attn[n, m, :]; zero-pad to RP
    attnT = np.zeros((EMB, RP), dtype=BF16NP)
    attnT[:, :R] = self_attn.transpose(2, 0, 1).reshape(EMB, R)

    # vwoh[p, t*N + j] = vw[m(r)] * (n(r) == j),  r = t*P + p
    vwoh = np.zeros((P, NTILE * N), dtype=np.float32)
    for t in range(NTILE):
        for p in range(min(P, R - t * P)):
            r = t * P + p
            vwoh[p, t * N + (r // M)] = value_w[r % M]
    vwoh = vwoh.astype(BF16NP)

    in_maps = []
    for c in range(NCORES):
        lo = c * LSH
        raw_c = np.zeros((RP, D, LP), dtype=BF16NP)
        raw_c[:R, :, :LSH] = raw_all[:, :, lo : lo + LSH]
        embT_c = np.zeros((EMB, LP), dtype=BF16NP)
        embT_c[:, :LSH] = embT_full[:, lo : lo + LSH]
        in_maps.append(
            {
                "raw": raw_c,
                "embT": embT_c,
                "attnT": attnT,
                "vwoh": vwoh,
            }
        )
    return in_maps


def _run(inputs, **spmd_kwargs):
    in_maps = _prep_in_maps(
        inputs["self_attn"], inputs["self_delta"], inputs["emb_table"], inputs["value_w"]
    )
    res = run_bass_kernel_spmd(
        _get_nc(), in_maps, core_ids=list(range(NCORES)), **spmd_kwargs
    )
    out = np.concatenate([r["out"] for r in res.results], axis=1)  # [N, L]
    return out, res


def kernel(**inputs) -> np.ndarray:
    out, _ = _run(inputs)
    return out


# revision 23
# speedup vs baseline: 1.0647x; 1.0647x over previous
"""Trainium2 Bass kernel: fused bmm+decay+reduce attention scorer (bf16).

Computes, for full inputs
    self_attn  [N=16, M=100, EMB=128] f32
    self_delta [N=16, M=100, L=10000, D=4] f32
    emb_table  [L+1=10001, EMB=128] f32
    value_w    [M=100] f32
the output
    out[n, l] = sum_m value_w[m] * (sum_d self_delta[n,m,l,d]) * (emb_table[1+l] . self_attn[n,m])
of shape [16, 10000] f32 (matches the reference jnp einsum chain).

Sharding: the candidate/location axis L is split 8 ways (1250 locations per
core); every core handles all 16 batch rows for its location range, so the
dominant self_delta stream is never replicated.

Structure (130 us f32 baseline -> 72 us max / 69 us mean across cores):
  *  The delta stream is staged host-side as bf16 (graded check is
     rel-err < 2e-2; measured impact ~4.6e-3), halving HBM traffic to
     16.7 MB/core, d-plane-major + l-padded to 1254 = 3*418 so every DVE
     operand is a packed 2-byte stream (DVE 2x perf mode).
  *  S = emb.attn is evacuated PSUM->SBUF with bf16 downcast on the ACT
     engine, so the decay multiply is an all-bf16 2x-mode op too.
  *  The raw stream lands in a fully resident 13-buffer pool; all DMAs
     are issued up-front on ONE HWDGE ring (SP) and sustain ~390 GB/s.
     A single in-order ring matters: alternating two rings was tried and
     every early tile arrived ~2x later (each ring runs at half rate),
     delaying the whole in-order compute pipeline.  DMA-CCE accumulate
     and GPSIMD adds were also tried: accumulate is slower AND wrong for
     this pattern; GPSIMD tensor ops collide with DVE 2-port perf mode.
  *  Consts are issued between the first raw tile and the rest on the
     same ring (they starve on a parallel ring -- v2 measured PE stalled
     to 17 us -- and ahead of all raw they delay the DVE start).
  *  Two-tile software pipeline: tile t's decay multiply is emitted
     after tile t+2's adds, so the in-order DVE queue (the saturated
     engine, ~41 us busy ~= the 43 us stream) never blocks on the
     PE->ACT S path.
  *  The LAST tile runs per 418-wide l-chunk: DMA, adds, multiply,
     acc-matmul, PSUM-evacuation and output DMA all chunk-pipelined,
     shrinking the post-stream drain tail to ~4 us.
Fixed overheads observed: ~6.3 us engine-barrier preamble (incl ~2.3 us
host start-event latency) and ~8.4 us NEFF semaphore-file reset
postamble; both are outside kernel control.  The device power-throttles
after back-to-back runs (throttle_active_nc0_time_ns in the profile;
per-op DVE durations inflate ~1.2x) -- benchmark from a cold device.
"""

import numpy as np
import ml_dtypes

import concourse.mybir as mybir
import concourse.tile as tile
from concourse import bacc
from concourse.bass_utils import run_bass_kernel_spmd

BF16NP = ml_dtypes.bfloat16

N, M, L, EMB, D = 16, 100, 10000, 128, 4
NCORES = 8
LSH = L // NCORES  # 1250 locations per core
LP = 1254          # padded to 3 * 418 (uniform PSUM-bank chunks, even widths)
CW = 418           # chunk width (<=512 f32 per PSUM bank)
NCHUNK = 3
R = N * M          # 1600 flattened (n, m) rows
P = 128
NTILE = (R + P - 1) // P  # 13 tiles; the last holds 64 rows
RP = NTILE * P            # 1664 padded rows
TILE_ORDER = [NTILE - 1] + list(range(NTILE - 1))  # half tile first (short ramp)
FP32 = mybir.dt.float32
BF16 = mybir.dt.bfloat16

_NC_CACHE = {}


def _build_nc():
    nc = bacc.Bacc(
        "TRN2", target_bir_lowering=False, debug=False, num_devices=NCORES
    )
    raw_d = nc.dram_tensor("raw", [RP, D, LP], BF16, kind="ExternalInput").ap()
    embT_d = nc.dram_tensor("embT", [EMB, LP], BF16, kind="ExternalInput").ap()
    attnT_d = nc.dram_tensor("attnT", [EMB, RP], BF16, kind="ExternalInput").ap()
    vwoh_d = nc.dram_tensor("vwoh", [P, NTILE * N], BF16, kind="ExternalInput").ap()
    out_d = nc.dram_tensor("out", [N, LSH], FP32, kind="ExternalOutput").ap()

    with tile.TileContext(nc) as tc:
        with (
            tc.tile_pool(name="const", bufs=1) as cpool,
            tc.tile_pool(name="raws", bufs=NTILE) as rpool,
            tc.tile_pool(name="a1p", bufs=2) as a1pool,
            tc.tile_pool(name="a2p", bufs=4) as a2pool,
            tc.tile_pool(name="ssb", bufs=4) as sbpool,
            tc.tile_pool(name="ptp", bufs=3) as ppool,
            tc.tile_pool(name="spsum", bufs=1, space="PSUM") as spool,
            tc.tile_pool(name="apsum", bufs=1, space="PSUM") as apool,
        ):
            # out accumulator rows n=0..15, one PSUM bank per l-chunk
            acc = apool.tile([N, NCHUNK, 512], FP32, tag="acc")

            # all raw-tile DMAs are independent of compute: issue the whole
            # stream up-front on one in-order ring so it never stalls and
            # tiles arrive in consumption order.  The first and last tiles
            # are transferred as three l-chunks to chunk-pipeline the
            # pipeline ramp and the drain tail.  The first two tiles go
            # BEFORE the consts (DVE needs raw ~3 us before PE needs consts).
            # only the last tile is l-chunked (chunk-pipelined drain tail);
            # chunking the leading tiles was tried and did not help
            chunked = {TILE_ORDER[-1]}
            raws = {}

            def emit_raw_dma(t):
                rows = min(P, R - t * P)
                raw_t = rpool.tile([P, D, LP], BF16, tag="raw")
                if t in chunked:
                    for j in range(NCHUNK):
                        nc.sync.dma_start(
                            out=raw_t[:rows, :, j * CW : (j + 1) * CW],
                            in_=raw_d[
                                t * P : t * P + rows, :, j * CW : (j + 1) * CW
                            ],
                        )
                else:
                    nc.sync.dma_start(
                        out=raw_t[:rows], in_=raw_d[t * P : t * P + rows]
                    )
                raws[t] = raw_t

            emit_raw_dma(TILE_ORDER[0])
            embT = cpool.tile([EMB, LP], BF16, tag="embT")
            nc.sync.dma_start(out=embT, in_=embT_d)
            attnT = cpool.tile([EMB, RP], BF16, tag="attnT")
            nc.sync.dma_start(out=attnT, in_=attnT_d)
            vwoh = cpool.tile([P, NTILE * N], BF16, tag="vwoh")
            nc.sync.dma_start(out=vwoh, in_=vwoh_d)
            for t in TILE_ORDER[1:]:
                emit_raw_dma(t)

            out_sb = cpool.tile([N, NCHUNK, CW], FP32, tag="out_sb")
            tiles = {}

            def emit_feed(t, adds_chunked):
                """S matmuls + ACT evacuation + the two D-sum adds."""
                rows = min(P, R - t * P)
                rv = raws[t]
                s_ps = spool.tile([P, NCHUNK, 512], FP32, tag="s")
                for j in range(NCHUNK):
                    nc.tensor.matmul(
                        s_ps[:rows, j, :CW],
                        attnT[:, t * P : t * P + rows],
                        embT[:, j * CW : (j + 1) * CW],
                        start=True,
                        stop=True,
                    )
                # evacuate + downcast on the ACT engine (keeps DVE free)
                s_sb = sbpool.tile([P, NCHUNK, CW], BF16, tag="ssb")
                nc.scalar.copy(out=s_sb[:rows], in_=s_ps[:rows, :, :CW])
                a1 = a1pool.tile([P, 2, LP], BF16, tag="a1")
                a2 = a2pool.tile([P, LP], BF16, tag="a2")
                p_t = ppool.tile([P, NCHUNK, CW], BF16, tag="p")
                tiles[t] = (rows, a2, s_sb, p_t)
                for j in range(NCHUNK) if adds_chunked else [None]:
                    emit_adds(t, rows, rv, a1, a2, j)

            def emit_adds(t, rows, rv, a1, a2, j):
                lsl = slice(None) if j is None else slice(j * CW, (j + 1) * CW)
                nc.vector.tensor_add(
                    out=a1[:rows, :, lsl],
                    in0=rv[:rows, 0:2, lsl],
                    in1=rv[:rows, 2:4, lsl],
                )
                nc.vector.tensor_add(
                    out=a2[:rows, lsl],
                    in0=a1[:rows, 0, lsl],
                    in1=a1[:rows, 1, lsl],
                )

            def emit_drain(t, ti, j):
                """Decay multiply + acc matmul(s) for tile t (chunk j or all)."""
                rows, a2, s_sb, p_t = tiles[t]
                a2v = a2.rearrange("p (c w) -> p c w", w=CW)
                csl = slice(None) if j is None else slice(j, j + 1)
                nc.vector.tensor_mul(
                    out=p_t[:rows, csl], in0=a2v[:rows, csl], in1=s_sb[:rows, csl]
                )
                last = ti == NTILE - 1
                for jj in range(NCHUNK) if j is None else [j]:
                    nc.tensor.matmul(
                        acc[:, jj, :CW],
                        vwoh[:rows, t * N : (t + 1) * N],
                        p_t[:rows, jj],
                        start=(ti == 0),
                        stop=last,
                    )
                    if last:
                        # chunk jj of acc is final: evacuate + store now,
                        # overlapped with the next chunk's DVE/PE work
                        w = min(LSH, (jj + 1) * CW) - jj * CW
                        nc.scalar.copy(out=out_sb[:, jj], in_=acc[:, jj, :CW])
                        nc.sync.dma_start(
                            out=out_d[:, jj * CW : jj * CW + w],
                            in_=out_sb[:, jj, :w],
                        )

            # two-tile software pipeline: tile t's multiply is emitted after
            # tile t+2's adds, so the in-order DVE queue never blocks on the
            # PE->ACT S path (measured 2.1 us stall at depth 1); the last
            # tile interleaves per chunk to keep the drain tail short
            KDEPTH = 2
            for ti, t in enumerate(TILE_ORDER):
                last = ti == NTILE - 1
                if not last:
                    emit_feed(t, adds_chunked=(t in chunked))
                    if ti >= KDEPTH:
                        emit_drain(TILE_ORDER[ti - KDEPTH], ti - KDEPTH, None)
                else:
                    for k in range(KDEPTH, 0, -1):
                        emit_drain(TILE_ORDER[ti - k], ti - k, None)
                    rows = min(P, R - t * P)
                    rv = raws[t]
                    s_ps = spool.tile([P, NCHUNK, 512], FP32, tag="s")
                    for j in range(NCHUNK):
                        nc.tensor.matmul(
                            s_ps[:rows, j, :CW],
                            attnT[:, t * P : t * P + rows],
                            embT[:, j * CW : (j + 1) * CW],
                            start=True,
                            stop=True,
                        )
                    s_sb = sbpool.tile([P, NCHUNK, CW], BF16, tag="ssb")
                    nc.scalar.copy(out=s_sb[:rows], in_=s_ps[:rows, :, :CW])
                    a1 = a1pool.tile([P, 2, LP], BF16, tag="a1")
                    a2 = a2pool.tile([P, LP], BF16, tag="a2")
                    p_t = ppool.tile([P, NCHUNK, CW], BF16, tag="p")
                    tiles[t] = (rows, a2, s_sb, p_t)
                    for j in range(NCHUNK):
                        emit_adds(t, rows, rv, a1, a2, j)
                        emit_drain(t, ti, j)

    nc.compile()
    return nc


def _get_nc():
    if "nc" not in _NC_CACHE:
        _NC_CACHE["nc"] = _build_nc()
    return _NC_CACHE["nc"]


def _prep_in_maps(self_attn, self_delta, emb_table, value_w):
    self_attn = np.asarray(self_attn, dtype=np.float32)
    self_delta = np.asarray(self_delta, dtype=np.float32)
    emb_table = np.asarray(emb_table, dtype=np.float32)
    value_w = np.asarray(value_w, dtype=np.float32)

    # [R, D, L] bf16, d-plane-major: one global transpose+cast, sliced per core
    raw_all = np.ascontiguousarray(
        self_delta.reshape(R, L, D).transpose(0, 2, 1)
    ).astype(BF16NP)

    embT_full = emb_table[1 : L + 1].T.astype(BF16NP)  # [EMB, L]

    # column r = n*M + m of attnT holds attn[n, m, :]; zero-pad to RP
    attnT = np.zeros((EMB, RP), dtype=BF16NP)
    attnT[:, :R] = self_attn.transpose(2, 0, 1).reshape(EMB, R)

    # vwoh[p, t*N + j] = vw[m(r)] * (n(r) == j),  r = t*P + p
    vwoh = np.zeros((P, NTILE * N), dtype=np.float32)
    for t in range(NTILE):
        for p in range(min(P, R - t * P)):
            r = t * P + p
            vwoh[p, t * N + (r // M)] = value_w[r % M]
    vwoh = vwoh.astype(BF16NP)

    in_maps = []
    for c in range(NCORES):
        lo = c * LSH
        raw_c = np.zeros((RP, D, LP), dtype=BF16NP)
        raw_c[:R, :, :LSH] = raw_all[:, :, lo : lo + LSH]
        embT_c = np.zeros((EMB, LP), dtype=BF16NP)
        embT_c[:, :LSH] = embT_full[:, lo : lo + LSH]
        in_maps.append(
            {
                "raw": raw_c,
                "embT": embT_c,
                "attnT": attnT,
                "vwoh": vwoh,
            }
        )
    return in_maps


def _run(inputs, **spmd_kwargs):
    in_maps = _prep_in_maps(
        inputs["self_attn"], inputs["self_delta"], inputs["emb_table"], inputs["value_w"]
    )
    res = run_bass_kernel_spmd(
        _get_nc(), in_maps, core_ids=list(range(NCORES)), **spmd_kwargs
    )
    out = np.concatenate([r["out"] for r in res.results], axis=1)  # [N, L]
    return out, res


def kernel(**inputs) -> np.ndarray:
    out, _ = _run(inputs)
    return out
